# revision 1
# baseline (speedup 1.0000x reference)
"""Memory-augmented attention kernel for Trainium2 (Bass/Tile), 8-core data parallel.

Reference computation (per row b of B=32768, D=512, K=5):
    q' = query@Wq + bq
    k  = mem@Wk + bk ; v = mem@Wv + bv
    scores = (q'.k_j)/sqrt(D) masked-softmax -> w
    mem_out = (sum_j w_j v_j)@Wo + bo
    gate = sigmoid([query, mem_out]@Wg + bg); conf = sigmoid(max_sim - 0.7)
    out = LN(query + gate*conf*mem_out) * ln_g + ln_b

Algebraic refactoring (all biases are zero and LN affine is identity in this
problem; a numpy fallback covers the general case):
    scores_bk = m_bk . (query_b @ (Wq @ Wk^T)) * scale
    mem_out_b = (sum_k w_bk m_bk) @ (Wv @ Wo)
    gate_b    = sigmoid(query_b . Wg[:D] + mcomb_b . (Wv@Wo@Wg[D:]))

Device mapping per 128-row tile (4-stage software pipeline, lag 3, so each
engine's in-order stream interleaves work from adjacent tiles):
    PE   : transpose q and mcomb (bf16), t = q@Wqk, mem = mcomb@Wvo, gate dots
           (all matmuls bf16 with fp32 PSUM accumulate; 1/sqrt(D) folded into
           Wqk on the host)
    DVE  : scores dot-products and the w-weighted memory combine via native
           scalar_tensor_tensor with accum_out (fp32), softmax glue, fused
           (mem*s)+q with free row-sum, LN scalar glue
    ACT  : exp / ln (rstd = exp(-0.5 ln(var+eps))), sigmoids via exp,
           PSUM->SBUF copies with bf16 casts, Square-acc for E[x^2], final LN
           apply. Only {Copy,Identity,Exp,Ln,Square} are used - one activation
           table, no table reloads.
    GPSIMD: q bf16 cast, mask penalty add, out-DMA via SWDGE

This container's walrus build only encodes one sync-wait per instruction and
cannot encode TENSOR_TENSOR_REDUCE / EVENT_SEMAPHORE_RANGE_CLEAR /
Pool-engine TensorScalarPtr; see _install_tile_patches and the single-dep
"touch" absorber ops below.
"""

import numpy as np

B, D, K = 32768, 512, 5
N_CORES = 8
ROWS = B // N_CORES        # rows per core
P = 128                    # partitions
NT_FULL = ROWS // P        # tiles per core (32)
NCH = D // P               # 128-contraction chunks (4)
SCALE = float(D) ** -0.5
BIG = 1.0e30
LN_EPS = 1e-5
SIM_THRESH = 0.7

_CACHE = {}

TRACE = False              # set by test harness to collect a HW profile
LAST_RESULTS = None        # BassKernelResults of the last run (for profiling)



def _install_tile_patches():
    """Work around two walrus limitations in this container:
    - instructions accept very few sync-wait slots: split the kernel-tail
      drain (which Tile loads with one wait per outstanding semaphore) into
      a chain of single-wait drains;
    - EVENT_SEMAPHORE_RANGE_CLEAR is not encodable: skip the on-device sem
      clear (each kernel() call executes a freshly loaded NEFF) while keeping
      the allocator bookkeeping.
    """
    import concourse.tile as tile
    from concourse.vector_clock import ScopedClock

    if getattr(tile.TileContext._drain_and_barrier, "_patched", False):
        return

    def patched(self, tick_clock, wait_clock):
        import bass_rust

        nc = self.nc
        drain_inst = nc.sync.drain()
        wait_clock.add_sem_waits(
            drain_inst.ins, ScopedClock({None: tick_clock.global_clock})
        )
        si = drain_inst.ins.sync_info
        waits = list(si.on_wait) if si is not None and si.on_wait else []
        if len(waits) > 1:
            drain_inst.ins.sync_info = bass_rust.SyncInfo(
                on_wait=waits[:1], on_update=list(si.on_update or [])
            )
            for w in waits[1:]:
                d2 = nc.sync.drain()
                d2.ins.sync_info = bass_rust.SyncInfo(on_wait=[w], on_update=[])
        nc.all_engine_barrier()
        assert self.sems is not None
        popped = nc._tile_sem_poison_stack.pop()
        assert popped is self._sem_poison
        sems = list(self.sems.allocated().values())
        sem_nums = [s.num for s in sems]
        nc._state.prepend_free_semaphores(sem_nums)
        for poison_set in nc._tile_sem_poison_stack:
            poison_set.update(sem_nums)
        nc.all_engine_barrier()

    patched._patched = True
    tile.TileContext._drain_and_barrier = patched

    # This walrus build accepts at most one sync-wait per instruction:
    # at commit time, peel off extra waits onto single-wait drain
    # instructions inserted just before the owner.
    _orig_commit = tile.TileContext._commit_instruction

    def commit_patched(self, inst, lazy_reg_writes=True):
        import bass_rust
        from concourse import mybir

        si = inst.sync_info
        if si is not None and si.on_wait and len(si.on_wait) > 1:
            waits = list(si.on_wait)
            inst.sync_info = bass_rust.SyncInfo(
                on_wait=waits[-1:], on_update=list(si.on_update or [])
            )
            for w in waits[:-1]:
                eng = self.nc.engines[inst.engine]
                if not hasattr(eng, "engine_nop"):
                    nop = mybir.InstDrain(
                        name=self.nc.get_next_instruction_name(), ins=[], outs=[]
                    )
                    nop.engine = inst.engine
                else:
                    # sequencer-only ENGINE_NOP: carries the wait without
                    # flushing the compute pipeline the way a drain does
                    nop = eng.engine_nop().ins
                nop.sync_info = bass_rust.SyncInfo(on_wait=[w], on_update=[])
                self._add_instruction(nop)
        return _orig_commit(self, inst, lazy_reg_writes)

    tile.TileContext._commit_instruction = commit_patched


def _build(ntiles=NT_FULL):
    import concourse.bass as bass
    import concourse.tile as tile
    from concourse import mybir

    _install_tile_patches()

    f32 = mybir.dt.float32
    f32r = mybir.dt.float32r
    bf16 = mybir.dt.bfloat16
    u8 = mybir.dt.uint8
    AF = mybir.ActivationFunctionType
    OP = mybir.AluOpType
    AX = mybir.AxisListType

    rows = ntiles * P
    rD = 1.0 / float(D)

    nc = bass.Bass()
    qm_d = nc.declare_dram_parameter("qm", [rows, (K + 1) * D], f32r, isOutput=False)
    sims_d = nc.declare_dram_parameter("sims", [rows, K], f32, isOutput=False)
    mask_d = nc.declare_dram_parameter("mask", [rows, K], u8, isOutput=False)
    wqk_d = nc.declare_dram_parameter("wqk", [D, D], bf16, isOutput=False)
    wvo_d = nc.declare_dram_parameter("wvo", [D, D], bf16, isOutput=False)
    gv_d = nc.declare_dram_parameter("gv", [D, 2], bf16, isOutput=False)
    id_d = nc.declare_dram_parameter("ident", [P, P], bf16, isOutput=False)
    idr_d = nc.declare_dram_parameter("identr", [P, P], f32, isOutput=False)
    o_d = nc.declare_dram_parameter("o", [rows, D], f32, isOutput=True)

    qm_t = qm_d.rearrange("(t p) d -> t p d", p=P)
    o_t = o_d.rearrange("(t p) d -> t p d", p=P)

    with tile.TileContext(nc) as tc:
        with (
            tc.tile_pool(name="consts", bufs=1) as consts,
            tc.tile_pool(name="qmload", bufs=6) as qmload,
            tc.tile_pool(name="work", bufs=3) as work,
            tc.tile_pool(name="smalls", bufs=6) as smalls,
            tc.tile_pool(name="pbig", bufs=5, space="PSUM") as pbig,
            tc.tile_pool(name="pmix", bufs=3, space="PSUM") as pmix,
        ):
            # ---- constants, loaded once ----
            wqk_sb = consts.tile([P, NCH, D], bf16)
            nc.sync.dma_start(out=wqk_sb, in_=wqk_d.rearrange("(c p) e -> p c e", p=P))
            wvo_sb = consts.tile([P, NCH, D], bf16)
            nc.sync.dma_start(out=wvo_sb, in_=wvo_d.rearrange("(c p) e -> p c e", p=P))
            g_sb = consts.tile([P, NCH, 2], bf16)
            nc.sync.dma_start(out=g_sb, in_=gv_d.rearrange("(c p) j -> p c j", p=P))
            ident = consts.tile([P, P], bf16)
            nc.sync.dma_start(out=ident, in_=id_d[:, :])
            identr = consts.tile([P, P], f32)
            nc.sync.dma_start(out=identr, in_=idr_d[:, :])

            sims_all = consts.tile([P, ntiles, K], f32)
            nc.sync.dma_start(
                out=sims_all, in_=sims_d.rearrange("(t p) k -> p t k", p=P)
            )
            mask_all = consts.tile([P, ntiles, K], u8)
            nc.sync.dma_start(
                out=mask_all, in_=mask_d.rearrange("(t p) k -> p t k", p=P)
            )

            thresh = consts.tile([P, 1], f32)
            nc.vector.memset(thresh, SIM_THRESH)
            epsc = consts.tile([P, 1], f32)
            nc.vector.memset(epsc, LN_EPS)

            # conf[b, t] = sigmoid(max_k sims - th) = 1/(1+exp(th - max))
            simmax = consts.tile([P, ntiles], f32)
            nc.vector.reduce_max(out=simmax, in_=sims_all, axis=AX.X)
            confe = consts.tile([P, ntiles], f32)
            nc.scalar.activation(
                out=confe, in_=simmax, func=AF.Exp, bias=thresh, scale=-1.0
            )
            confe1 = consts.tile([P, ntiles], f32)
            nc.vector.tensor_scalar(
                out=confe1, in0=confe, scalar1=1.0, scalar2=None, op0=OP.add
            )
            conf_all = consts.tile([P, ntiles], f32)
            nc.vector.reciprocal(out=conf_all, in_=confe1)

            # pen[b, t, k] = 0 if valid else -BIG
            m01 = consts.tile([P, ntiles, K], f32)
            nc.vector.tensor_copy(out=m01, in_=mask_all)
            pen_all = consts.tile([P, ntiles, K], f32)
            nc.vector.tensor_scalar(
                out=pen_all, in0=m01, scalar1=1.0, scalar2=BIG,
                op0=OP.subtract, op1=OP.mult,
            )

            actabs = consts.tile([P, 2], f32)
            nc.vector.memset(actabs, 0.0)

            def touch_dve(ap):
                tt = smalls.tile([P, 2], f32, tag="dvet", name="dvet")
                nc.vector.tensor_copy(out=tt[:, 0:ap.free_size()], in_=ap)

            def touch_gp(ap):
                tt = smalls.tile([P, 2], f32, tag="gpt", name="gpt")
                nc.gpsimd.tensor_copy(out=tt[:, 0:ap.free_size()], in_=ap)

            def touch_act(ap):
                tt = smalls.tile([P, 2], f32, tag="actt", name="actt")
                nc.scalar.copy(out=tt[:, 0:ap.free_size()], in_=ap)

            # Per-tile live state, keyed by tile index. Three-stage software
            # pipeline (lag 2) so each engine's in-order stream interleaves
            # work from adjacent tiles instead of idling through each tile's
            # serial dependency chain.
            st = {}

            def dma_in(t):
                s = st.setdefault(t, {})
                qm = qmload.tile([P, (K + 1) * D], f32r, tag="qm", name="qmtile")
                nc.sync.dma_start(out=qm, in_=qm_t[t])
                s["qmr"] = qm
                s["q"] = qm[:, 0:D].bitcast(f32)
                s["m"] = qm[:, D:].bitcast(f32)

            def stage_a(t):
                # qT via PE transpose (bf16); t = q@Wqk ; nqdot = -(q.g1)
                s = st[t]
                q_bf = work.tile([P, D], bf16, tag="q_bf")
                touch_gp(s["q"][:, 0:2])
                nc.gpsimd.tensor_copy(out=q_bf, in_=s["q"])
                psum_q = pmix.tile([P, D], bf16, tag="pmix")
                for c in range(NCH):
                    sl = slice(c * P, (c + 1) * P)
                    nc.tensor.transpose(psum_q[:, sl], q_bf[:, sl], ident)
                qT = work.tile([P, D], bf16, tag="qT")
                nc.scalar.copy(out=qT, in_=psum_q)

                s["pt"] = pbig.tile([P, D], f32, tag="pbig", name="pt")
                psum_qg = pmix.tile([P, 1], f32, tag="pmix")
                for c in range(NCH):
                    sl = slice(c * P, (c + 1) * P)
                    nc.tensor.matmul(
                        s["pt"],
                        lhsT=qT[:, sl],
                        rhs=wqk_sb[:, c, :],
                        start=(c == 0), stop=(c == NCH - 1),
                    )
                for c in range(NCH):
                    sl = slice(c * P, (c + 1) * P)
                    nc.tensor.matmul(
                        psum_qg,
                        lhsT=qT[:, sl],
                        rhs=g_sb[:, c, 0:1],
                        start=(c == 0), stop=(c == NCH - 1),
                    )
                s["nqdot"] = smalls.tile([P, 1], f32, tag="nqdot", name="nqdot")
                nc.scalar.activation(
                    out=s["nqdot"], in_=psum_qg, func=AF.Copy, scale=-1.0
                )

            def stage_b(t):
                # scores_k = pen_k + (m_k . t)   (1/sqrt(D) folded into Wqk)
                s = st[t]
                raw = smalls.tile([P, K], f32, tag="rawsc", name="rawsc")
                scratch = work.tile([P, D], f32, tag="scratch")
                touch_dve(s["m"][:, 0:2])
                touch_dve(s["pt"][:, 0:2])
                for k in range(K):
                    nc.vector.scalar_tensor_tensor(
                        out=scratch,
                        in0=s["m"][:, k * D:(k + 1) * D],
                        scalar=1.0,
                        in1=s["pt"],
                        op0=OP.mult, op1=OP.mult,
                        accum_out=raw[:, k:k + 1],
                    )
                s["scores"] = smalls.tile([P, K], f32, tag="scores", name="scores")
                nc.gpsimd.tensor_tensor(
                    out=s["scores"], in0=raw, in1=pen_all[:, t, :], op=OP.add
                )
                s["negrmax"] = smalls.tile([P, 1], f32, tag="negrmax", name="negrmax")
                nc.vector.reduce_max(
                    out=s["negrmax"], in_=s["scores"], axis=AX.X, negate=True
                )

            def stage_c1(t):
                # w = exp(scores - max); unnormalized mcomb' = sum_k w_k m_k;
                # mem' = mcomb'@Wvo ; mdot' = mcomb'.g2 ; rsum = 1/sumexp
                s = st[t]
                s["w"] = smalls.tile([P, K], f32, tag="w", name="wtile")
                sumexp = smalls.tile([P, 1], f32, tag="sumexp", name="sumexp")
                touch_act(s["scores"][:, 0:2])
                nc.scalar.activation(
                    out=s["w"], in_=s["scores"], func=AF.Exp,
                    bias=s["negrmax"], scale=1.0, accum_out=sumexp,
                )
                s["rsum"] = smalls.tile([P, 1], f32, tag="rsum", name="rsum")
                nc.vector.reciprocal(out=s["rsum"], in_=sumexp)
                s["negrsum"] = smalls.tile([P, 1], f32, tag="negrsum", name="negrsum")
                nc.vector.tensor_scalar(
                    out=s["negrsum"], in0=s["rsum"], scalar1=-1.0,
                    scalar2=None, op0=OP.mult,
                )
                # mcomb = sum_k w_k m_k  via diag(w_k) matmuls (fp32r PE)
                touch_dve(s["w"][:, 0:2])
                psum_mc = pbig.tile([P, D], f32, tag="pbig")
                for k in range(K):
                    dk = smalls.tile([P, P], f32r, tag="diag", name="diag")
                    nc.vector.tensor_scalar(
                        out=dk, in0=identr, scalar1=s["w"][:, k:k + 1],
                        scalar2=None, op0=OP.mult,
                    )
                    nc.tensor.matmul(
                        psum_mc,
                        lhsT=dk,
                        rhs=s["qmr"][:, (k + 1) * D:(k + 2) * D],
                        start=(k == 0), stop=(k == K - 1),
                    )
                mcomb_bf = work.tile([P, D], bf16, tag="mcomb_bf")
                touch_act(psum_mc[:, 0:2])
                nc.scalar.copy(out=mcomb_bf, in_=psum_mc)

                psum_mt = pmix.tile([P, D], bf16, tag="pmix")
                for c in range(NCH):
                    sl = slice(c * P, (c + 1) * P)
                    nc.tensor.transpose(psum_mt[:, sl], mcomb_bf[:, sl], ident)
                mcT = work.tile([P, D], bf16, tag="mcT")
                nc.scalar.copy(out=mcT, in_=psum_mt)

                s["pmem"] = pbig.tile([P, D], f32, tag="pbig", name="pmem")
                psum_mg = pmix.tile([P, 1], f32, tag="pmix")
                for c in range(NCH):
                    sl = slice(c * P, (c + 1) * P)
                    nc.tensor.matmul(
                        s["pmem"],
                        lhsT=mcT[:, sl],
                        rhs=wvo_sb[:, c, :],
                        start=(c == 0), stop=(c == NCH - 1),
                    )
                for c in range(NCH):
                    sl = slice(c * P, (c + 1) * P)
                    nc.tensor.matmul(
                        psum_mg,
                        lhsT=mcT[:, sl],
                        rhs=g_sb[:, c, 1:2],
                        start=(c == 0), stop=(c == NCH - 1),
                    )
                s["mdot"] = smalls.tile([P, 1], f32, tag="mdot", name="mdot")
                nc.scalar.copy(out=s["mdot"], in_=psum_mg)

            def stage_c2(t):
                # s = conf*rsum/(1+exp(-(qdot + rsum*mdot'))) ;
                # out_pre = s*mem' + q ; layernorm ; store
                s = st.pop(t)
                touch_act(s["negrsum"][:, 0:1])
                ge = smalls.tile([P, 1], f32, tag="ge")
                nc.scalar.activation(
                    out=ge, in_=s["mdot"], func=AF.Exp,
                    bias=s["nqdot"], scale=s["negrsum"],
                )
                gp1 = smalls.tile([P, 1], f32, tag="gp1")
                nc.vector.tensor_scalar(
                    out=gp1, in0=ge, scalar1=1.0, scalar2=None, op0=OP.add
                )
                rgp = smalls.tile([P, 1], f32, tag="rgp")
                nc.vector.reciprocal(out=rgp, in_=gp1)
                s_sb = smalls.tile([P, 1], f32, tag="s")
                nc.vector.tensor_scalar(
                    out=s_sb, in0=rgp, scalar1=conf_all[:, t:t + 1],
                    scalar2=s["rsum"], op0=OP.mult, op1=OP.mult,
                )

                touch_dve(s["pmem"][:, 0:2])
                touch_dve(s_sb[:, 0:1])
                out_pre = work.tile([P, D], f32, tag="out_pre")
                rowsum = smalls.tile([P, 1], f32, tag="rowsum")
                nc.vector.scalar_tensor_tensor(
                    out=out_pre, in0=s["pmem"], scalar=s_sb, in1=s["q"],
                    op0=OP.mult, op1=OP.add, accum_out=rowsum,
                )

                sumsq = smalls.tile([P, 1], f32, tag="sumsq")
                sqscr = work.tile([P, D], f32, tag="sqscr")
                nc.scalar.activation(
                    out=sqscr, in_=out_pre, func=AF.Square, accum_out=sumsq
                )
                mu = smalls.tile([P, 1], f32, tag="mu")
                nc.vector.tensor_scalar(
                    out=mu, in0=rowsum, scalar1=rD, scalar2=None, op0=OP.mult
                )
                mu2 = smalls.tile([P, 1], f32, tag="mu2")
                nc.gpsimd.tensor_tensor(out=mu2, in0=mu, in1=mu, op=OP.mult)
                varc = smalls.tile([P, 1], f32, tag="varc")
                nc.vector.scalar_tensor_tensor(
                    out=varc, in0=sumsq, scalar=rD, in1=mu2,
                    op0=OP.mult, op1=OP.subtract,
                )
                lnv = smalls.tile([P, 1], f32, tag="lnv")
                nc.scalar.activation(
                    out=lnv, in_=varc, func=AF.Ln, bias=epsc, scale=1.0
                )
                rstd = smalls.tile([P, 1], f32, tag="rstd")
                nc.scalar.activation(out=rstd, in_=lnv, func=AF.Exp, scale=-0.5)
                nmr = smalls.tile([P, 1], f32, tag="nmr")
                nc.vector.tensor_scalar(
                    out=nmr, in0=mu, scalar1=rstd, scalar2=-1.0,
                    op0=OP.mult, op1=OP.mult,
                )
                out_sb = work.tile([P, D], f32, tag="out_sb")
                touch_act(nmr[:, 0:1])
                nc.scalar.memzero(out_sb[:, 0:2])
                nc.scalar.activation(
                    out=out_sb, in_=out_pre, func=AF.Identity, scale=rstd, bias=nmr
                )
                nc.gpsimd.dma_start(out=o_t[t], in_=out_sb)

            dma_in(0)
            for i in range(ntiles + 3):
                if i + 1 < ntiles:
                    dma_in(i + 1)
                if i < ntiles:
                    stage_a(i)
                if 0 <= i - 3:
                    stage_c2(i - 3)
                if 0 <= i - 2 <= ntiles - 1:
                    stage_c1(i - 2)
                if 0 <= i - 1 <= ntiles - 1:
                    stage_b(i - 1)

    return nc


def _numpy_fallback(query, retrieved_memories, similarities, mask,
                    Wq, bq, Wk, bk, Wv, bv, Wo, bo, Wg, bg, ln_g, ln_b):
    x = query.astype(np.float64)
    m = retrieved_memories.astype(np.float64)
    q = x @ Wq + bq
    k = np.einsum("bkd,de->bke", m, Wk.astype(np.float64)) + bk
    v = np.einsum("bkd,de->bke", m, Wv.astype(np.float64)) + bv
    scores = np.einsum("bd,bkd->bk", q, k) * (D ** -0.5)
    scores = np.where(mask, scores, -np.inf)
    sm = scores - scores.max(-1, keepdims=True)
    w = np.exp(sm)
    w /= w.sum(-1, keepdims=True)
    w = np.where(mask, w, 0.0)
    mem = np.einsum("bk,bkd->bd", w, v) @ Wo + bo
    gate = 1 / (1 + np.exp(-(np.concatenate([x, mem], -1) @ Wg + bg)))
    conf = 1 / (1 + np.exp(-(similarities.max(-1, keepdims=True) - SIM_THRESH)))
    out = x + (gate * conf) * mem
    mu = out.mean(-1, keepdims=True)
    var = ((out - mu) ** 2).mean(-1, keepdims=True)
    out = (out - mu) / np.sqrt(var + LN_EPS) * ln_g + ln_b
    return out.astype(np.float32)


def kernel(**inputs):
    global LAST_RESULTS
    query = np.ascontiguousarray(np.asarray(inputs["query"], dtype=np.float32))
    mem = np.ascontiguousarray(
        np.asarray(inputs["retrieved_memories"], dtype=np.float32)
    )
    sims = np.ascontiguousarray(np.asarray(inputs["similarities"], dtype=np.float32))
    mask = np.asarray(inputs["mask"])
    Wq = np.asarray(inputs["Wq"], dtype=np.float64)
    Wk = np.asarray(inputs["Wk"], dtype=np.float64)
    Wv = np.asarray(inputs["Wv"], dtype=np.float64)
    Wo = np.asarray(inputs["Wo"], dtype=np.float64)
    Wg = np.asarray(inputs["Wg"], dtype=np.float64)

    # The device kernel folds all-zero biases / identity LN affine away.
    nontrivial = (
        any(np.any(np.asarray(inputs[n])) for n in ("bq", "bk", "bv", "bo", "bg"))
        or np.any(np.asarray(inputs["ln_b"]))
        or np.any(np.asarray(inputs["ln_g"]) != 1.0)
    )
    if nontrivial or query.shape != (B, D):
        return _numpy_fallback(
            query, mem, sims, mask, Wq=Wq, bq=np.asarray(inputs["bq"]),
            Wk=Wk, bk=np.asarray(inputs["bk"]), Wv=Wv, bv=np.asarray(inputs["bv"]),
            Wo=Wo, bo=np.asarray(inputs["bo"]), Wg=Wg, bg=np.asarray(inputs["bg"]),
            ln_g=np.asarray(inputs["ln_g"]), ln_b=np.asarray(inputs["ln_b"]),
        )

    import ml_dtypes
    bf = ml_dtypes.bfloat16
    wqk = np.ascontiguousarray(((Wq @ Wk.T) * (float(D) ** -0.5)).astype(bf))
    wvo64 = Wv @ Wo
    wvo = np.ascontiguousarray(wvo64.astype(bf))
    g1 = Wg[:D, 0]
    g2 = wvo64 @ Wg[D:, 0]
    gv = np.ascontiguousarray(np.stack([g1, g2], axis=1).astype(bf))
    ident = np.eye(P, dtype=bf)
    identr = np.eye(P, dtype=np.float32)

    if "nc" not in _CACHE:
        _CACHE["nc"] = _build()
    nc = _CACHE["nc"]

    qm = np.concatenate([query, mem.reshape(B, K * D)], axis=1)
    mask_u8 = np.ascontiguousarray(mask.astype(np.uint8))
    in_maps = []
    for c in range(N_CORES):
        sl = slice(c * ROWS, (c + 1) * ROWS)
        in_maps.append({
            "qm": qm[sl], "sims": sims[sl], "mask": mask_u8[sl],
            "wqk": wqk, "wvo": wvo, "gv": gv, "ident": ident, "identr": identr,
        })

    from concourse.bass_utils import run_bass_kernel_spmd

    res = run_bass_kernel_spmd(nc, in_maps, list(range(N_CORES)), trace=TRACE)
    LAST_RESULTS = res
    return np.concatenate([res.results[c]["o"] for c in range(N_CORES)], axis=0)



# revision 5
# speedup vs baseline: 1.1313x; 1.1313x over previous
"""Memory-augmented attention kernel for Trainium2 (Bass/Tile), 8-core data parallel.

Reference computation (per row b of B=32768, D=512, K=5):
    q' = query@Wq + bq
    k  = mem@Wk + bk ; v = mem@Wv + bv
    scores = (q'.k_j)/sqrt(D) masked-softmax -> w
    mem_out = (sum_j w_j v_j)@Wo + bo
    gate = sigmoid([query, mem_out]@Wg + bg); conf = sigmoid(max_sim - 0.7)
    out = LN(query + gate*conf*mem_out) * ln_g + ln_b

Algebraic refactoring (biases are zero / LN affine identity in this problem;
a numpy fallback covers the general case):
    scores_bk = m_bk . (query_b @ (Wq @ Wk^T) / sqrt(D))
    mem_out_b = (sum_k w_bk m_bk) @ (Wv @ Wo)
    gate_b    = sigmoid(query_b . Wg[:D] + mcomb_b . (Wv@Wo@Wg[D:]))
    conf      = sigmoid(max_k sims - 0.7)  (computed on host)

All bulk data moves HBM<->SBUF in bf16 (query+memories staged as one bf16
buffer, output stored bf16 and upcast on host), halving DMA traffic vs f32.
Scores are computed without max-subtraction (|scores| ~ N(0,1), exp safe).

Per 128-row tile, stages pipelined with deep lag so every engine streams:
    IN : DMA qm tile (bf16) + qT via SBUF->SBUF xbar dma_start_transpose
    A  : PE  pt = q@Wqk (lhsT=qT chunks), [-qdot, qsum/D] via gq cols;
         ACT copies pt (bf16), -qdot, qsum to SBUF
    B  : DVE scores dots (scalar_tensor_tensor bf16 + accum), ACT exp,
         DVE masked-sum STT -> w,sumexp, recip, 5x diag_k = I*w_k (bf16)
    C  : PE  mcomb = sum_k diag_k^T @ m_k (unnormalized); ACT copy bf16;
         DMA xbar transpose mcomb -> mcT
    D  : PE  mem = mcomb@Wvo, [-mdot, rowsum(Wvo)-dot] via gm cols;
         ACT ge = exp(-rsum*mdot - qdot) reading PSUM; DVE gp1, rgp
    E1 : DVE s = rgp*conf*rsum; ACT mem_s = s*mem (PSUM->SBUF bf16), memsum
    E2 : GPSIMD out_pre = mem_s + q (bf16)
    E3 : DVE mu; GPSIMD mu2; ACT sumsq (Square+accum); DVE var
    E4 : ACT lnv, rstd (exp(-0.5 ln(var+eps))); DVE nmr, apply (tensor_scalar
         2-AP-scalar, bf16 4x); GPSIMD out DMA (SWDGE)

This container's walrus build only encodes one sync-wait per instruction and
cannot encode EVENT_SEMAPHORE_RANGE_CLEAR; see _install_tile_patches.
"""

import numpy as np

B, D, K = 32768, 512, 5
N_CORES = 8
ROWS = B // N_CORES        # rows per core
P = 128                    # partitions
NT_FULL = ROWS // P        # tiles per core (32)
NCH = D // P               # 128-contraction chunks (4)
LN_EPS = 1e-5
SIM_THRESH = 0.7

_CACHE = {}

TRACE = False              # set by test harness to collect a HW profile
LAST_RESULTS = None        # BassKernelResults of the last run (for profiling)


def _install_tile_patches():
    """Work around two walrus limitations in this container:
    - instructions accept very few sync-wait slots: split the kernel-tail
      drain (which Tile loads with one wait per outstanding semaphore) into
      a chain of single-wait drains;
    - EVENT_SEMAPHORE_RANGE_CLEAR is not encodable: skip the on-device sem
      clear (each kernel() call executes a freshly loaded NEFF) while keeping
      the allocator bookkeeping.
    """
    import concourse.tile as tile
    from concourse.vector_clock import ScopedClock

    if getattr(tile.TileContext._drain_and_barrier, "_patched", False):
        return

    def patched(self, tick_clock, wait_clock):
        import bass_rust

        nc = self.nc
        drain_inst = nc.sync.drain()
        wait_clock.add_sem_waits(
            drain_inst.ins, ScopedClock({None: tick_clock.global_clock})
        )
        si = drain_inst.ins.sync_info
        waits = list(si.on_wait) if si is not None and si.on_wait else []
        if len(waits) > 1:
            drain_inst.ins.sync_info = bass_rust.SyncInfo(
                on_wait=waits[:1], on_update=list(si.on_update or [])
            )
            for w in waits[1:]:
                d2 = nc.sync.drain()
                d2.ins.sync_info = bass_rust.SyncInfo(on_wait=[w], on_update=[])
        nc.all_engine_barrier()
        assert self.sems is not None
        popped = nc._tile_sem_poison_stack.pop()
        assert popped is self._sem_poison
        sems = list(self.sems.allocated().values())
        sem_nums = [s.num for s in sems]
        nc._state.prepend_free_semaphores(sem_nums)
        for poison_set in nc._tile_sem_poison_stack:
            poison_set.update(sem_nums)
        nc.all_engine_barrier()

    patched._patched = True
    tile.TileContext._drain_and_barrier = patched

    # This walrus build accepts at most one sync-wait per instruction:
    # at commit time, peel off extra waits onto single-wait nop/drain
    # instructions inserted just before the owner.
    _orig_commit = tile.TileContext._commit_instruction

    def commit_patched(self, inst, lazy_reg_writes=True):
        import bass_rust
        from concourse import mybir

        si = inst.sync_info
        if si is not None and si.on_wait and len(si.on_wait) > 1:
            waits = list(si.on_wait)
            inst.sync_info = bass_rust.SyncInfo(
                on_wait=waits[-1:], on_update=list(si.on_update or [])
            )
            for w in waits[:-1]:
                eng = self.nc.engines[inst.engine]
                if not hasattr(eng, "engine_nop"):
                    nop = mybir.InstDrain(
                        name=self.nc.get_next_instruction_name(), ins=[], outs=[]
                    )
                    nop.engine = inst.engine
                else:
                    # sequencer-only ENGINE_NOP: carries the wait without
                    # flushing the compute pipeline the way a drain does
                    nop = eng.engine_nop().ins
                nop.sync_info = bass_rust.SyncInfo(on_wait=[w], on_update=[])
                self._add_instruction(nop)
        return _orig_commit(self, inst, lazy_reg_writes)

    tile.TileContext._commit_instruction = commit_patched


def _build(ntiles=NT_FULL):
    import concourse.bass as bass
    import concourse.tile as tile
    from concourse import mybir

    _install_tile_patches()

    f32 = mybir.dt.float32
    bf16 = mybir.dt.bfloat16
    AF = mybir.ActivationFunctionType
    OP = mybir.AluOpType

    rows = ntiles * P
    rD = 1.0 / float(D)

    nc = bass.Bass()
    qm_d = nc.declare_dram_parameter("qm", [rows, (K + 1) * D], bf16, isOutput=False)
    mask_d = nc.declare_dram_parameter("maskf", [rows, K], f32, isOutput=False)
    conf_d = nc.declare_dram_parameter("conf", [rows, 1], f32, isOutput=False)
    wqk_d = nc.declare_dram_parameter("wqk", [D, D], bf16, isOutput=False)
    wvo_d = nc.declare_dram_parameter("wvo", [D, D], bf16, isOutput=False)
    gq_d = nc.declare_dram_parameter("gq", [D, 2], bf16, isOutput=False)
    gm_d = nc.declare_dram_parameter("gm", [D, 2], bf16, isOutput=False)
    id_d = nc.declare_dram_parameter("ident", [P, P], bf16, isOutput=False)
    o_d = nc.declare_dram_parameter("o", [rows, D], bf16, isOutput=True)

    qm_t = qm_d.rearrange("(t p) x -> t p x", p=P)
    o_t = o_d.rearrange("(t p) d -> t p d", p=P)

    with tile.TileContext(nc) as tc:
        with (
            tc.tile_pool(name="consts", bufs=1) as consts,
            tc.tile_pool(name="qmload", bufs=12) as qmload,
            tc.tile_pool(name="tload", bufs=4) as tload,
            tc.tile_pool(name="work", bufs=4) as work,
            tc.tile_pool(name="smalls", bufs=12) as smalls,
            tc.tile_pool(name="ppt", bufs=2, space="PSUM") as ppt,
            tc.tile_pool(name="pmc", bufs=2, space="PSUM") as pmc,
            tc.tile_pool(name="pmem", bufs=2, space="PSUM") as pmem,
            tc.tile_pool(name="pmix", bufs=2, space="PSUM") as pmix,
        ):
            # ---- constants, loaded once ----
            wqk_sb = consts.tile([P, NCH, D], bf16)
            nc.sync.dma_start(out=wqk_sb, in_=wqk_d.rearrange("(c p) e -> p c e", p=P))
            wvo_sb = consts.tile([P, NCH, D], bf16)
            nc.sync.dma_start(out=wvo_sb, in_=wvo_d.rearrange("(c p) e -> p c e", p=P))
            gq_sb = consts.tile([P, NCH, 2], bf16)
            nc.sync.dma_start(out=gq_sb, in_=gq_d.rearrange("(c p) j -> p c j", p=P))
            gm_sb = consts.tile([P, NCH, 2], bf16)
            nc.sync.dma_start(out=gm_sb, in_=gm_d.rearrange("(c p) j -> p c j", p=P))
            identb = consts.tile([P, P], bf16)
            nc.sync.dma_start(out=identb, in_=id_d[:, :])
            mask_all = consts.tile([P, ntiles, K], f32)
            nc.sync.dma_start(
                out=mask_all, in_=mask_d.rearrange("(t p) k -> p t k", p=P)
            )
            conf_all = consts.tile([P, ntiles], f32)
            nc.sync.dma_start(
                out=conf_all, in_=conf_d.rearrange("(t p) j -> p (t j)", p=P)
            )
            epsc = consts.tile([P, 1], f32)
            nc.vector.memset(epsc, LN_EPS)

            def touch_dve(ap):
                tt = smalls.tile([P, 2], f32, tag="dvet", name="dvet")
                nc.vector.tensor_copy(out=tt[:, 0:ap.free_size()], in_=ap)

            def touch_act(ap):
                tt = smalls.tile([P, 2], f32, tag="actt", name="actt")
                nc.scalar.copy(out=tt[:, 0:ap.free_size()], in_=ap)

            def touch_gp(ap):
                tt = smalls.tile([P, 2], f32, tag="gpt", name="gpt")
                nc.gpsimd.tensor_copy(out=tt[:, 0:ap.free_size()], in_=ap)

            # Per-tile live state, keyed by tile index. Deep software pipeline
            # so each engine's in-order stream interleaves work from many
            # tiles instead of idling through each tile's dependency chain.
            st = {}

            def stage_in(t):
                s = st.setdefault(t, {})
                qm = qmload.tile([P, K + 1, D], bf16, tag="qm", name="qmtile")
                nc.sync.dma_start(out=qm, in_=qm_t[t].rearrange("p (s d) -> p s d", d=D))
                qT = tload.tile([P, NCH, P], bf16, tag="qT", name="qT")
                nc.sync.dma_start_transpose(out=qT, in_=qm[:, 0, :])
                s["qm"] = qm
                s["qT"] = qT

            def stage_a(t):
                # pt = q@Wqk ; q2 = [-qdot, qsum/D]
                s = st[t]
                pt_ps = ppt.tile([P, D], f32, tag="pt", name="pt_ps")
                for c in range(NCH):
                    nc.tensor.matmul(
                        pt_ps, lhsT=s["qT"][:, c, :], rhs=wqk_sb[:, c, :],
                        start=(c == 0), stop=(c == NCH - 1),
                    )
                q2_ps = pmix.tile([P, 2], f32, tag="mix2", name="q2_ps")
                for c in range(NCH):
                    nc.tensor.matmul(
                        q2_ps, lhsT=s["qT"][:, c, :], rhs=gq_sb[:, c, :],
                        start=(c == 0), stop=(c == NCH - 1),
                    )
                s["pt"] = work.tile([P, D], bf16, tag="pt_sb", name="pt_sb")
                nc.scalar.copy(out=s["pt"], in_=pt_ps)
                s["nqdot"] = smalls.tile([P, 1], f32, tag="nqdot", name="nqdot")
                nc.scalar.copy(out=s["nqdot"], in_=q2_ps[:, 0:1])
                s["qsum"] = smalls.tile([P, 1], f32, tag="qsum", name="qsum")
                nc.scalar.copy(out=s["qsum"], in_=q2_ps[:, 1:2])

            def stage_b(t):
                # raw_k = m_k . pt ; w = exp(raw)*mask ; rsum = 1/sum(w)
                s = st[t]
                raw = smalls.tile([P, K], f32, tag="raw", name="raw")
                scratch = work.tile([P, D], bf16, tag="scratch", name="scratch")
                touch_dve(s["qm"][:, 1, 0:2])
                touch_dve(s["pt"][:, 0:2])
                for k in range(K):
                    nc.vector.scalar_tensor_tensor(
                        out=scratch, in0=s["qm"][:, 1 + k, :], scalar=1.0,
                        in1=s["pt"], op0=OP.mult, op1=OP.mult,
                        accum_out=raw[:, k:k + 1],
                    )
                expw = smalls.tile([P, K], f32, tag="expw", name="expw")
                touch_act(raw[:, 0:2])
                nc.scalar.activation(out=expw, in_=raw, func=AF.Exp)
                s["w"] = smalls.tile([P, K], f32, tag="w", name="w")
                sumexp = smalls.tile([P, 1], f32, tag="sumexp", name="sumexp")
                nc.vector.scalar_tensor_tensor(
                    out=s["w"], in0=expw, scalar=1.0, in1=mask_all[:, t, :],
                    op0=OP.mult, op1=OP.mult, accum_out=sumexp,
                )
                s["rsum"] = smalls.tile([P, 1], f32, tag="rsum", name="rsum")
                nc.vector.reciprocal(out=s["rsum"], in_=sumexp)
                diag = work.tile([P, K, P], bf16, tag="diag", name="diag")
                for k in range(K):
                    nc.vector.tensor_scalar(
                        out=diag[:, k, :], in0=identb,
                        scalar1=s["w"][:, k:k + 1], scalar2=None, op0=OP.mult,
                    )
                s["diag"] = diag

            def stage_c(t):
                # mcomb = sum_k w_k m_k (unnormalized); mcT via xbar transpose
                s = st[t]
                mc_ps = pmc.tile([P, D], f32, tag="mc", name="mc_ps")
                for k in range(K):
                    nc.tensor.matmul(
                        mc_ps, lhsT=s["diag"][:, k, :], rhs=s["qm"][:, 1 + k, :],
                        start=(k == 0), stop=(k == K - 1),
                    )
                mcb = work.tile([P, D], bf16, tag="mcb", name="mcb")
                nc.scalar.copy(out=mcb, in_=mc_ps)
                mcT = tload.tile([P, NCH, P], bf16, tag="mcT", name="mcT")
                nc.sync.dma_start_transpose(out=mcT, in_=mcb)
                s["mcT"] = mcT

            def stage_d(t):
                # mem = mcomb@Wvo ; m2 = [-mdot, mcomb.rowsum(Wvo)/D] ;
                # ge = exp(-rsum*mdot - qdot) ; rgp = sigmoid
                s = st[t]
                mem_ps = pmem.tile([P, D], f32, tag="mem", name="mem_ps")
                for c in range(NCH):
                    nc.tensor.matmul(
                        mem_ps, lhsT=s["mcT"][:, c, :], rhs=wvo_sb[:, c, :],
                        start=(c == 0), stop=(c == NCH - 1),
                    )
                m2_ps = pmix.tile([P, 2], f32, tag="mix2", name="m2_ps")
                for c in range(NCH):
                    nc.tensor.matmul(
                        m2_ps, lhsT=s["mcT"][:, c, :], rhs=gm_sb[:, c, :],
                        start=(c == 0), stop=(c == NCH - 1),
                    )
                s["mem_ps"] = mem_ps
                ge = smalls.tile([P, 1], f32, tag="ge", name="ge")
                touch_act(s["rsum"][:, 0:1])
                nc.scalar.activation(
                    out=ge, in_=m2_ps[:, 0:1], func=AF.Exp,
                    bias=s["nqdot"], scale=s["rsum"],
                )
                s["memsum"] = smalls.tile([P, 1], f32, tag="memsum", name="memsum")
                nc.scalar.copy(out=s["memsum"], in_=m2_ps[:, 1:2])
                gp1 = smalls.tile([P, 1], f32, tag="gp1", name="gp1")
                nc.vector.tensor_scalar(
                    out=gp1, in0=ge, scalar1=1.0, scalar2=None, op0=OP.add
                )
                s["rgp"] = smalls.tile([P, 1], f32, tag="rgp", name="rgp")
                nc.vector.reciprocal(out=s["rgp"], in_=gp1)

            def stage_e1(t):
                # s = conf*rsum*sigmoid ; mem_s = s*mem (PSUM->SBUF bf16)
                s = st[t]
                s_sb = smalls.tile([P, 1], f32, tag="s_sb", name="s_sb")
                nc.vector.tensor_scalar(
                    out=s_sb, in0=s["rgp"], scalar1=conf_all[:, t:t + 1],
                    scalar2=s["rsum"], op0=OP.mult, op1=OP.mult,
                )
                s["s_sb"] = s_sb
                mem_s = work.tile([P, D], bf16, tag="mem_s", name="mem_s")
                touch_act(s_sb[:, 0:1])
                nc.scalar.activation(
                    out=mem_s, in_=s["mem_ps"], func=AF.Copy, scale=s_sb
                )
                s["mem_s"] = mem_s

            def stage_e2(t):
                # out_pre = mem_s + q  (GPSIMD, bf16)
                s = st[t]
                out_pre = work.tile([P, D], bf16, tag="out_pre", name="out_pre")
                touch_gp(s["mem_s"][:, 0:2])
                nc.gpsimd.tensor_tensor(
                    out=out_pre, in0=s["mem_s"], in1=s["qm"][:, 0, :], op=OP.add
                )
                s["out_pre"] = out_pre

            def stage_e3(t):
                # mu = qsum/D + s*memsum ; sumsq = sum(out_pre^2) ;
                # var = sumsq/D - mu^2
                s = st[t]
                mu = smalls.tile([P, 1], f32, tag="mu", name="mu")
                nc.vector.scalar_tensor_tensor(
                    out=mu, in0=s["memsum"], scalar=s["s_sb"], in1=s["qsum"],
                    op0=OP.mult, op1=OP.add,
                )
                s["mu"] = mu
                mu2 = smalls.tile([P, 1], f32, tag="mu2", name="mu2")
                nc.gpsimd.tensor_tensor(out=mu2, in0=mu, in1=mu, op=OP.mult)
                sumsq = smalls.tile([P, 1], f32, tag="sumsq", name="sumsq")
                sqscr = work.tile([P, D], bf16, tag="sqscr", name="sqscr")
                touch_act(s["out_pre"][:, 0:2])
                nc.scalar.activation(
                    out=sqscr, in_=s["out_pre"], func=AF.Square, accum_out=sumsq
                )
                varc = smalls.tile([P, 1], f32, tag="varc", name="varc")
                nc.vector.scalar_tensor_tensor(
                    out=varc, in0=sumsq, scalar=rD, in1=mu2,
                    op0=OP.mult, op1=OP.subtract,
                )
                s["varc"] = varc

            def stage_e4(t):
                # rstd = exp(-0.5 ln(var+eps)) ; out = (out_pre - mu)*rstd
                s = st.pop(t)
                lnv = smalls.tile([P, 1], f32, tag="lnv", name="lnv")
                nc.scalar.activation(
                    out=lnv, in_=s["varc"], func=AF.Ln, bias=epsc, scale=1.0
                )
                rstd = smalls.tile([P, 1], f32, tag="rstd", name="rstd")
                nc.scalar.activation(out=rstd, in_=lnv, func=AF.Exp, scale=-0.5)
                nmr = smalls.tile([P, 1], f32, tag="nmr", name="nmr")
                nc.vector.tensor_scalar(
                    out=nmr, in0=s["mu"], scalar1=rstd, scalar2=-1.0,
                    op0=OP.mult, op1=OP.mult,
                )
                out_sb = work.tile([P, D], bf16, tag="out_sb", name="out_sb")
                touch_dve(nmr[:, 0:1])
                nc.vector.tensor_scalar(
                    out=out_sb, in0=s["out_pre"], scalar1=rstd,
                    scalar2=nmr, op0=OP.mult, op1=OP.add,
                )
                touch_gp(out_sb[:, 0:2])
                nc.gpsimd.dma_start(out=o_t[t], in_=out_sb)

            stages = [
                (0, stage_in), (2, stage_a), (3, stage_b), (4, stage_c),
                (6, stage_d), (7, stage_e1), (8, stage_e2), (9, stage_e3),
                (10, stage_e4),
            ]
            total_lag = stages[-1][0]
            for i in range(ntiles + total_lag):
                for lag, fn in stages:
                    t = i - lag
                    if 0 <= t < ntiles:
                        fn(t)

    return nc


def _numpy_fallback(query, retrieved_memories, similarities, mask,
                    Wq, bq, Wk, bk, Wv, bv, Wo, bo, Wg, bg, ln_g, ln_b):
    x = query.astype(np.float64)
    m = retrieved_memories.astype(np.float64)
    q = x @ Wq + bq
    k = np.einsum("bkd,de->bke", m, Wk.astype(np.float64)) + bk
    v = np.einsum("bkd,de->bke", m, Wv.astype(np.float64)) + bv
    scores = np.einsum("bd,bkd->bk", q, k) * (D ** -0.5)
    scores = np.where(mask, scores, -np.inf)
    sm = scores - scores.max(-1, keepdims=True)
    w = np.exp(sm)
    w /= w.sum(-1, keepdims=True)
    w = np.where(mask, w, 0.0)
    mem = np.einsum("bk,bkd->bd", w, v) @ Wo + bo
    gate = 1 / (1 + np.exp(-(np.concatenate([x, mem], -1) @ Wg + bg)))
    conf = 1 / (1 + np.exp(-(similarities.max(-1, keepdims=True) - SIM_THRESH)))
    out = x + (gate * conf) * mem
    mu = out.mean(-1, keepdims=True)
    var = ((out - mu) ** 2).mean(-1, keepdims=True)
    out = (out - mu) / np.sqrt(var + LN_EPS) * ln_g + ln_b
    return out.astype(np.float32)


def kernel(**inputs):
    global LAST_RESULTS
    query = np.asarray(inputs["query"], dtype=np.float32)
    mem = np.asarray(inputs["retrieved_memories"], dtype=np.float32)
    sims = np.asarray(inputs["similarities"], dtype=np.float32)
    mask = np.asarray(inputs["mask"])
    Wq = np.asarray(inputs["Wq"], dtype=np.float64)
    Wk = np.asarray(inputs["Wk"], dtype=np.float64)
    Wv = np.asarray(inputs["Wv"], dtype=np.float64)
    Wo = np.asarray(inputs["Wo"], dtype=np.float64)
    Wg = np.asarray(inputs["Wg"], dtype=np.float64)

    # The device kernel folds all-zero biases / identity LN affine away.
    nontrivial = (
        any(np.any(np.asarray(inputs[n])) for n in ("bq", "bk", "bv", "bo", "bg"))
        or np.any(np.asarray(inputs["ln_b"]))
        or np.any(np.asarray(inputs["ln_g"]) != 1.0)
    )
    if nontrivial or query.shape != (B, D):
        return _numpy_fallback(
            query, mem, sims, mask, Wq=Wq, bq=np.asarray(inputs["bq"]),
            Wk=Wk, bk=np.asarray(inputs["bk"]), Wv=Wv, bv=np.asarray(inputs["bv"]),
            Wo=Wo, bo=np.asarray(inputs["bo"]), Wg=Wg, bg=np.asarray(inputs["bg"]),
            ln_g=np.asarray(inputs["ln_g"]), ln_b=np.asarray(inputs["ln_b"]),
        )

    import ml_dtypes
    bf = ml_dtypes.bfloat16
    wqk = np.ascontiguousarray(((Wq @ Wk.T) * (float(D) ** -0.5)).astype(bf))
    wvo64 = Wv @ Wo
    wvo = np.ascontiguousarray(wvo64.astype(bf))
    g1 = Wg[:D, 0]
    g2 = wvo64 @ Wg[D:, 0]
    ones_rD = np.full(D, 1.0 / D)
    gq = np.ascontiguousarray(np.stack([-g1, ones_rD], axis=1).astype(bf))
    gm = np.ascontiguousarray(
        np.stack([-g2, wvo64.sum(axis=1) / D], axis=1).astype(bf)
    )
    ident = np.eye(P, dtype=bf)

    if "nc" not in _CACHE:
        _CACHE["nc"] = _build()
    nc = _CACHE["nc"]

    qm = np.empty((B, (K + 1) * D), dtype=bf)
    qm[:, :D] = query
    qm[:, D:] = mem.reshape(B, K * D)
    maskf = np.ascontiguousarray(mask.astype(np.float32))
    conf = (1.0 / (1.0 + np.exp(-(sims.max(axis=-1) - SIM_THRESH)))).astype(
        np.float32
    ).reshape(B, 1)
    in_maps = []
    for c in range(N_CORES):
        sl = slice(c * ROWS, (c + 1) * ROWS)
        in_maps.append({
            "qm": qm[sl], "maskf": maskf[sl], "conf": conf[sl],
            "wqk": wqk, "wvo": wvo, "gq": gq, "gm": gm, "ident": ident,
        })

    from concourse.bass_utils import run_bass_kernel_spmd

    res = run_bass_kernel_spmd(nc, in_maps, list(range(N_CORES)), trace=TRACE)
    LAST_RESULTS = res
    return np.concatenate(
        [res.results[c]["o"].astype(np.float32) for c in range(N_CORES)], axis=0
    )


# revision 13
# speedup vs baseline: 1.1869x; 1.0492x over previous
"""Memory-augmented attention kernel for Trainium2 (Bass/Tile), 8-core data parallel.

Reference computation (per row b of B=32768, D=512, K=5):
    q' = query@Wq + bq
    k  = mem@Wk + bk ; v = mem@Wv + bv
    scores = (q'.k_j)/sqrt(D) masked-softmax -> w
    mem_out = (sum_j w_j v_j)@Wo + bo
    gate = sigmoid([query, mem_out]@Wg + bg); conf = sigmoid(max_sim - 0.7)
    out = LN(query + gate*conf*mem_out) * ln_g + ln_b

Algebraic refactoring (biases are zero / LN affine identity in this problem;
a numpy fallback covers the general case):
    scores_bk = m_bk . (query_b @ (Wq @ Wk^T) / sqrt(D))
    mem_out_b = (sum_k w_bk m_bk) @ (Wv @ Wo)
    gate_b    = sigmoid(query_b . Wg[:D] + mcomb_b . (Wv@Wo@Wg[D:]))
    conf      = sigmoid(max_k sims - 0.7)  (computed on host)

All bulk data moves HBM<->SBUF in bf16 (query+memories staged as one bf16
buffer, output stored bf16 and upcast on host), halving DMA traffic vs f32.
Scores are computed without max-subtraction (|scores| ~ N(0,1), exp safe).

Per 128-row tile, stages pipelined with deep lag so every engine streams:
    IN : DMA qm tile (bf16) + qT via SBUF->SBUF xbar dma_start_transpose
    A  : PE  pt = q@Wqk (lhsT=qT chunks), [-qdot, qsum/D] via gq cols;
         ACT copies pt (bf16), -qdot, qsum to SBUF
    B  : DVE scores dots (scalar_tensor_tensor bf16 + accum), ACT exp,
         DVE masked-sum STT -> w,sumexp, recip, 5x diag_k = I*w_k (bf16)
    C  : PE  mcomb = sum_k diag_k^T @ m_k (unnormalized); ACT copy bf16;
         DMA xbar transpose mcomb -> mcT
    D  : PE  mem = mcomb@Wvo, [-mdot, rowsum(Wvo)-dot] via gm cols;
         ACT ge = exp(-rsum*mdot - qdot) reading PSUM; DVE gp1, rgp
    E1 : DVE s = rgp*conf*rsum; ACT mem_s = s*mem (PSUM->SBUF bf16), memsum
    E2 : GPSIMD out_pre = mem_s + q (bf16)
    E3 : DVE mu; GPSIMD mu2; ACT sumsq (Square+accum); DVE var
    E4 : ACT lnv, rstd (exp(-0.5 ln(var+eps))); DVE nmr, apply (tensor_scalar
         2-AP-scalar, bf16 4x); GPSIMD out DMA (SWDGE)

This container's walrus build only encodes one sync-wait per instruction and
cannot encode EVENT_SEMAPHORE_RANGE_CLEAR; see _install_tile_patches.
"""

import numpy as np

B, D, K = 32768, 512, 5
N_CORES = 8
ROWS = B // N_CORES        # rows per core
P = 128                    # partitions
NT_FULL = ROWS // P        # tiles per core (32)
NCH = D // P               # 128-contraction chunks (4)
LN_EPS = 1e-5
SIM_THRESH = 0.7

_CACHE = {}

TRACE = False              # set by test harness to collect a HW profile
LAST_RESULTS = None        # BassKernelResults of the last run (for profiling)


def _install_tile_patches():
    """Work around two walrus limitations in this container:
    - instructions accept very few sync-wait slots: split the kernel-tail
      drain (which Tile loads with one wait per outstanding semaphore) into
      a chain of single-wait drains;
    - EVENT_SEMAPHORE_RANGE_CLEAR is not encodable: skip the on-device sem
      clear (each kernel() call executes a freshly loaded NEFF) while keeping
      the allocator bookkeeping.
    """
    import concourse.tile as tile
    from concourse.vector_clock import ScopedClock

    if getattr(tile.TileContext._drain_and_barrier, "_patched", False):
        return

    def patched(self, tick_clock, wait_clock):
        import bass_rust

        nc = self.nc
        drain_inst = nc.sync.drain()
        wait_clock.add_sem_waits(
            drain_inst.ins, ScopedClock({None: tick_clock.global_clock})
        )
        si = drain_inst.ins.sync_info
        waits = list(si.on_wait) if si is not None and si.on_wait else []
        if len(waits) > 1:
            drain_inst.ins.sync_info = bass_rust.SyncInfo(
                on_wait=waits[:1], on_update=list(si.on_update or [])
            )
            for w in waits[1:]:
                d2 = nc.sync.drain()
                d2.ins.sync_info = bass_rust.SyncInfo(on_wait=[w], on_update=[])
        nc.all_engine_barrier()
        assert self.sems is not None
        popped = nc._tile_sem_poison_stack.pop()
        assert popped is self._sem_poison
        sems = list(self.sems.allocated().values())
        sem_nums = [s.num for s in sems]
        nc._state.prepend_free_semaphores(sem_nums)
        for poison_set in nc._tile_sem_poison_stack:
            poison_set.update(sem_nums)
        nc.all_engine_barrier()

    patched._patched = True
    tile.TileContext._drain_and_barrier = patched

    # This walrus build accepts at most one sync-wait per instruction:
    # at commit time, peel off extra waits onto single-wait nop/drain
    # instructions inserted just before the owner.
    _orig_commit = tile.TileContext._commit_instruction

    def commit_patched(self, inst, lazy_reg_writes=True):
        import bass_rust
        from concourse import mybir

        si = inst.sync_info
        if si is not None and si.on_wait and len(si.on_wait) > 1:
            waits = list(si.on_wait)
            inst.sync_info = bass_rust.SyncInfo(
                on_wait=waits[-1:], on_update=list(si.on_update or [])
            )
            for w in waits[:-1]:
                eng = self.nc.engines[inst.engine]
                if not hasattr(eng, "engine_nop"):
                    nop = mybir.InstDrain(
                        name=self.nc.get_next_instruction_name(), ins=[], outs=[]
                    )
                    nop.engine = inst.engine
                else:
                    # sequencer-only ENGINE_NOP: carries the wait without
                    # flushing the compute pipeline the way a drain does
                    nop = eng.engine_nop().ins
                nop.sync_info = bass_rust.SyncInfo(on_wait=[w], on_update=[])
                self._add_instruction(nop)
        return _orig_commit(self, inst, lazy_reg_writes)

    tile.TileContext._commit_instruction = commit_patched


def _build(ntiles=NT_FULL):
    import concourse.bass as bass
    import concourse.tile as tile
    from concourse import mybir

    _install_tile_patches()

    f32 = mybir.dt.float32
    bf16 = mybir.dt.bfloat16
    AF = mybir.ActivationFunctionType
    OP = mybir.AluOpType

    rows = ntiles * P
    rD = 1.0 / float(D)

    nc = bass.Bass()
    qm_d = nc.declare_dram_parameter("qm", [rows, (K + 1) * D], bf16, isOutput=False)
    mask_d = nc.declare_dram_parameter("maskf", [rows, K], f32, isOutput=False)
    conf_d = nc.declare_dram_parameter("conf", [rows, 1], f32, isOutput=False)
    wqk_d = nc.declare_dram_parameter("wqk", [D, D], bf16, isOutput=False)
    wvo_d = nc.declare_dram_parameter("wvo", [D, D], bf16, isOutput=False)
    gq_d = nc.declare_dram_parameter("gq", [D, 2], bf16, isOutput=False)
    gm_d = nc.declare_dram_parameter("gm", [D, 2], bf16, isOutput=False)
    id_d = nc.declare_dram_parameter("ident", [P, P], bf16, isOutput=False)
    o_d = nc.declare_dram_parameter("o", [rows, D], bf16, isOutput=True)

    qm_t = qm_d.rearrange("(t p) x -> t p x", p=P)
    o_t = o_d.rearrange("(t p) d -> t p d", p=P)

    with tile.TileContext(nc) as tc:
        with (
            tc.tile_pool(name="consts", bufs=1) as consts,
            tc.tile_pool(name="qmload", bufs=12) as qmload,
            tc.tile_pool(name="tload", bufs=6) as tload,
            tc.tile_pool(name="work", bufs=4) as work,
            tc.tile_pool(name="smalls", bufs=12) as smalls,
            tc.tile_pool(name="ppt", bufs=2, space="PSUM") as ppt,
            tc.tile_pool(name="pmc", bufs=2, space="PSUM") as pmc,
            tc.tile_pool(name="pmem", bufs=2, space="PSUM") as pmem,
            tc.tile_pool(name="pmix", bufs=2, space="PSUM") as pmix,
        ):
            # ---- constants, loaded once ----
            wqk_sb = consts.tile([P, NCH, D], bf16)
            nc.sync.dma_start(out=wqk_sb, in_=wqk_d.rearrange("(c p) e -> p c e", p=P))
            wvo_sb = consts.tile([P, NCH, D], bf16)
            nc.sync.dma_start(out=wvo_sb, in_=wvo_d.rearrange("(c p) e -> p c e", p=P))
            gq_sb = consts.tile([P, NCH, 2], bf16)
            nc.sync.dma_start(out=gq_sb, in_=gq_d.rearrange("(c p) j -> p c j", p=P))
            gm_sb = consts.tile([P, NCH, 2], bf16)
            nc.sync.dma_start(out=gm_sb, in_=gm_d.rearrange("(c p) j -> p c j", p=P))
            identb = consts.tile([P, P], bf16)
            nc.sync.dma_start(out=identb, in_=id_d[:, :])
            mask_all = consts.tile([P, ntiles, K], f32)
            nc.sync.dma_start(
                out=mask_all, in_=mask_d.rearrange("(t p) k -> p t k", p=P)
            )
            conf_all = consts.tile([P, ntiles], f32)
            nc.sync.dma_start(
                out=conf_all, in_=conf_d.rearrange("(t p) j -> p (t j)", p=P)
            )
            epsc = consts.tile([P, 1], f32)
            nc.vector.memset(epsc, LN_EPS)

            def touch_dve(ap):
                tt = smalls.tile([P, 2], f32, tag="dvet", name="dvet")
                nc.vector.tensor_copy(out=tt[:, 0:ap.free_size()], in_=ap)

            def touch_act(ap):
                tt = smalls.tile([P, 2], f32, tag="actt", name="actt")
                nc.scalar.copy(out=tt[:, 0:ap.free_size()], in_=ap)

            def touch_gp(ap):
                tt = smalls.tile([P, 2], f32, tag="gpt", name="gpt")
                nc.gpsimd.tensor_copy(out=tt[:, 0:ap.free_size()], in_=ap)

            # Per-tile live state, keyed by tile index. Deep software pipeline
            # so each engine's in-order stream interleaves work from many
            # tiles instead of idling through each tile's dependency chain.
            st = {}

            def stage_in(t):
                s = st.setdefault(t, {})
                qm = qmload.tile([P, K + 1, D], bf16, tag="qm", name="qmtile")
                nc.sync.dma_start(out=qm, in_=qm_t[t].rearrange("p (s d) -> p s d", d=D))
                qT = tload.tile([P, NCH, P], bf16, tag="qT", name="qT")
                nc.sync.dma_start_transpose(out=qT, in_=qm[:, 0, :])
                s["qm"] = qm
                s["qT"] = qT

            def stage_a(t):
                # pt = q@Wqk ; q2 = [-qdot, qsum/D]
                s = st[t]
                pt_ps = ppt.tile([P, D], f32, tag="pt", name="pt_ps")
                for c in range(NCH):
                    nc.tensor.matmul(
                        pt_ps, lhsT=s["qT"][:, c, :], rhs=wqk_sb[:, c, :],
                        start=(c == 0), stop=(c == NCH - 1),
                    )
                q2_ps = pmix.tile([P, 2], f32, tag="mix2", name="q2_ps")
                for c in range(NCH):
                    nc.tensor.matmul(
                        q2_ps, lhsT=s["qT"][:, c, :], rhs=gq_sb[:, c, :],
                        start=(c == 0), stop=(c == NCH - 1),
                    )
                s["pt"] = work.tile([P, D], bf16, tag="pt_sb", name="pt_sb")
                nc.scalar.copy(out=s["pt"], in_=pt_ps)
                q2sb = smalls.tile([P, 2], f32, tag="q2sb", name="q2sb")
                nc.scalar.copy(out=q2sb, in_=q2_ps)
                s["nqdot"] = q2sb[:, 0:1]
                s["qsum"] = q2sb[:, 1:2]

            def stage_b(t):
                # raw_k = m_k . pt ; w = exp(raw)*mask ; rsum = 1/sum(w)
                s = st[t]
                raw = smalls.tile([P, K], f32, tag="raw", name="raw")
                scratch = work.tile([P, D], bf16, tag="scratch", name="scratch")
                touch_dve(s["qm"][:, 1, 0:2])
                touch_dve(s["pt"][:, 0:2])
                for k in range(K):
                    nc.vector.scalar_tensor_tensor(
                        out=scratch, in0=s["qm"][:, 1 + k, :], scalar=1.0,
                        in1=s["pt"], op0=OP.mult, op1=OP.mult,
                        accum_out=raw[:, k:k + 1],
                    )
                expw = smalls.tile([P, K], f32, tag="expw", name="expw")
                nc.scalar.activation(out=expw, in_=raw, func=AF.Exp)
                s["w"] = smalls.tile([P, K], bf16, tag="w", name="w")
                sumexp = smalls.tile([P, 1], f32, tag="sumexp", name="sumexp")
                nc.vector.scalar_tensor_tensor(
                    out=s["w"], in0=expw, scalar=1.0, in1=mask_all[:, t, :],
                    op0=OP.mult, op1=OP.mult, accum_out=sumexp,
                )
                s["rsum"] = smalls.tile([P, 1], f32, tag="rsum", name="rsum")
                nc.vector.reciprocal(out=s["rsum"], in_=sumexp)
                diag = work.tile([P, K, P], bf16, tag="diag", name="diag")
                nc.vector.tensor_tensor(
                    out=diag,
                    in0=identb[:, None, :].broadcast_to([P, K, P]),
                    in1=s["w"][:, :, None].broadcast_to([P, K, P]),
                    op=OP.mult,
                )
                s["diag"] = diag

            def stage_c(t):
                # mcomb = sum_k w_k m_k (unnormalized); mcT via xbar transpose
                s = st[t]
                mc_ps = pmc.tile([P, D], f32, tag="mc", name="mc_ps")
                for k in range(K):
                    nc.tensor.matmul(
                        mc_ps, lhsT=s["diag"][:, k, :], rhs=s["qm"][:, 1 + k, :],
                        start=(k == 0), stop=(k == K - 1),
                    )
                mcb = work.tile([P, D], bf16, tag="mcb", name="mcb")
                nc.scalar.copy(out=mcb, in_=mc_ps)
                mcT = tload.tile([P, NCH, P], bf16, tag="mcT", name="mcT")
                nc.sync.dma_start_transpose(out=mcT, in_=mcb)
                s["mcT"] = mcT

            def stage_d(t):
                # mem = mcomb@Wvo ; m2 = [-mdot, mcomb.rowsum(Wvo)/D] ;
                # ge = exp(-rsum*mdot - qdot) ; rgp = sigmoid
                s = st[t]
                mem_ps = pmem.tile([P, D], f32, tag="mem", name="mem_ps")
                for c in range(NCH):
                    nc.tensor.matmul(
                        mem_ps, lhsT=s["mcT"][:, c, :], rhs=wvo_sb[:, c, :],
                        start=(c == 0), stop=(c == NCH - 1),
                    )
                m2_ps = pmix.tile([P, 2], f32, tag="mix2", name="m2_ps")
                for c in range(NCH):
                    nc.tensor.matmul(
                        m2_ps, lhsT=s["mcT"][:, c, :], rhs=gm_sb[:, c, :],
                        start=(c == 0), stop=(c == NCH - 1),
                    )
                s["mem_ps"] = mem_ps
                m2sb = smalls.tile([P, 2], f32, tag="m2sb", name="m2sb")
                nc.scalar.copy(out=m2sb, in_=m2_ps)
                s["memsum"] = m2sb[:, 1:2]
                ge = smalls.tile([P, 1], f32, tag="ge", name="ge")
                touch_act(s["rsum"][:, 0:1])
                nc.scalar.activation(
                    out=ge, in_=m2sb[:, 0:1], func=AF.Exp,
                    bias=s["nqdot"], scale=s["rsum"],
                )
                gp1 = smalls.tile([P, 1], f32, tag="gp1", name="gp1")
                nc.vector.tensor_scalar(
                    out=gp1, in0=ge, scalar1=1.0, scalar2=None, op0=OP.add
                )
                s["rgp"] = smalls.tile([P, 1], f32, tag="rgp", name="rgp")
                nc.vector.reciprocal(out=s["rgp"], in_=gp1)

            def stage_e1(t):
                # s = conf*rsum*sigmoid ; mem_s = s*mem (PSUM->SBUF bf16)
                s = st[t]
                s_sb = smalls.tile([P, 1], f32, tag="s_sb", name="s_sb")
                nc.vector.tensor_scalar(
                    out=s_sb, in0=s["rgp"], scalar1=conf_all[:, t:t + 1],
                    scalar2=s["rsum"], op0=OP.mult, op1=OP.mult,
                )
                s["s_sb"] = s_sb
                mem_s = work.tile([P, D], bf16, tag="mem_s", name="mem_s")
                touch_act(s_sb[:, 0:1])
                nc.scalar.activation(
                    out=mem_s, in_=s["mem_ps"], func=AF.Copy, scale=s_sb
                )
                s["mem_s"] = mem_s

            def stage_e2(t):
                # out_pre = mem_s + q  (GPSIMD, bf16)
                s = st[t]
                out_pre = work.tile([P, D], bf16, tag="out_pre", name="out_pre")
                touch_gp(s["mem_s"][:, 0:2])
                nc.gpsimd.tensor_tensor(
                    out=out_pre, in0=s["mem_s"], in1=s["qm"][:, 0, :], op=OP.add
                )
                s["out_pre"] = out_pre

            def stage_e3(t):
                # mu = qsum/D + s*memsum ; sumsq = sum(out_pre^2) ;
                # var = sumsq/D - mu^2
                s = st[t]
                mu = smalls.tile([P, 1], f32, tag="mu", name="mu")
                nc.vector.scalar_tensor_tensor(
                    out=mu, in0=s["memsum"], scalar=s["s_sb"], in1=s["qsum"],
                    op0=OP.mult, op1=OP.add,
                )
                s["mu"] = mu
                mu2 = smalls.tile([P, 1], f32, tag="mu2", name="mu2")
                nc.gpsimd.tensor_tensor(out=mu2, in0=mu, in1=mu, op=OP.mult)
                sumsq = smalls.tile([P, 1], f32, tag="sumsq", name="sumsq")
                sqscr = work.tile([P, D], bf16, tag="sqscr", name="sqscr")
                touch_act(s["out_pre"][:, 0:2])
                nc.scalar.activation(
                    out=sqscr, in_=s["out_pre"], func=AF.Square, accum_out=sumsq
                )
                varc = smalls.tile([P, 1], f32, tag="varc", name="varc")
                nc.vector.scalar_tensor_tensor(
                    out=varc, in0=sumsq, scalar=rD, in1=mu2,
                    op0=OP.mult, op1=OP.subtract,
                )
                s["varc"] = varc

            def stage_e4(t):
                # rstd = exp(-0.5 ln(var+eps)) ; out = (out_pre - mu)*rstd
                s = st.pop(t)
                lnv = smalls.tile([P, 1], f32, tag="lnv", name="lnv")
                nc.scalar.activation(
                    out=lnv, in_=s["varc"], func=AF.Ln, bias=epsc, scale=1.0
                )
                rstd = smalls.tile([P, 1], f32, tag="rstd", name="rstd")
                nc.scalar.activation(out=rstd, in_=lnv, func=AF.Exp, scale=-0.5)
                nmr = smalls.tile([P, 1], f32, tag="nmr", name="nmr")
                nc.vector.tensor_scalar(
                    out=nmr, in0=s["mu"], scalar1=rstd, scalar2=-1.0,
                    op0=OP.mult, op1=OP.mult,
                )
                out_sb = work.tile([P, D], bf16, tag="out_sb", name="out_sb")
                nc.vector.tensor_scalar(
                    out=out_sb, in0=s["out_pre"], scalar1=rstd,
                    scalar2=nmr, op0=OP.mult, op1=OP.add,
                )
                touch_gp(out_sb[:, 0:2])
                nc.gpsimd.dma_start(out=o_t[t], in_=out_sb)

            stages = [
                (0, stage_in), (2, stage_a), (3, stage_b), (4, stage_c),
                (7, stage_d), (8, stage_e1), (9, stage_e2), (10, stage_e3),
                (11, stage_e4),
            ]
            total_lag = stages[-1][0]
            for i in range(ntiles + total_lag):
                for lag, fn in stages:
                    t = i - lag
                    if 0 <= t < ntiles:
                        fn(t)

    return nc


def _numpy_fallback(query, retrieved_memories, similarities, mask,
                    Wq, bq, Wk, bk, Wv, bv, Wo, bo, Wg, bg, ln_g, ln_b):
    x = query.astype(np.float64)
    m = retrieved_memories.astype(np.float64)
    q = x @ Wq + bq
    k = np.einsum("bkd,de->bke", m, Wk.astype(np.float64)) + bk
    v = np.einsum("bkd,de->bke", m, Wv.astype(np.float64)) + bv
    scores = np.einsum("bd,bkd->bk", q, k) * (D ** -0.5)
    scores = np.where(mask, scores, -np.inf)
    sm = scores - scores.max(-1, keepdims=True)
    w = np.exp(sm)
    w /= w.sum(-1, keepdims=True)
    w = np.where(mask, w, 0.0)
    mem = np.einsum("bk,bkd->bd", w, v) @ Wo + bo
    gate = 1 / (1 + np.exp(-(np.concatenate([x, mem], -1) @ Wg + bg)))
    conf = 1 / (1 + np.exp(-(similarities.max(-1, keepdims=True) - SIM_THRESH)))
    out = x + (gate * conf) * mem
    mu = out.mean(-1, keepdims=True)
    var = ((out - mu) ** 2).mean(-1, keepdims=True)
    out = (out - mu) / np.sqrt(var + LN_EPS) * ln_g + ln_b
    return out.astype(np.float32)


def kernel(**inputs):
    global LAST_RESULTS
    query = np.asarray(inputs["query"], dtype=np.float32)
    mem = np.asarray(inputs["retrieved_memories"], dtype=np.float32)
    sims = np.asarray(inputs["similarities"], dtype=np.float32)
    mask = np.asarray(inputs["mask"])
    Wq = np.asarray(inputs["Wq"], dtype=np.float64)
    Wk = np.asarray(inputs["Wk"], dtype=np.float64)
    Wv = np.asarray(inputs["Wv"], dtype=np.float64)
    Wo = np.asarray(inputs["Wo"], dtype=np.float64)
    Wg = np.asarray(inputs["Wg"], dtype=np.float64)

    # The device kernel folds all-zero biases / identity LN affine away.
    nontrivial = (
        any(np.any(np.asarray(inputs[n])) for n in ("bq", "bk", "bv", "bo", "bg"))
        or np.any(np.asarray(inputs["ln_b"]))
        or np.any(np.asarray(inputs["ln_g"]) != 1.0)
    )
    if nontrivial or query.shape != (B, D):
        return _numpy_fallback(
            query, mem, sims, mask, Wq=Wq, bq=np.asarray(inputs["bq"]),
            Wk=Wk, bk=np.asarray(inputs["bk"]), Wv=Wv, bv=np.asarray(inputs["bv"]),
            Wo=Wo, bo=np.asarray(inputs["bo"]), Wg=Wg, bg=np.asarray(inputs["bg"]),
            ln_g=np.asarray(inputs["ln_g"]), ln_b=np.asarray(inputs["ln_b"]),
        )

    import ml_dtypes
    bf = ml_dtypes.bfloat16
    wqk = np.ascontiguousarray(((Wq @ Wk.T) * (float(D) ** -0.5)).astype(bf))
    wvo64 = Wv @ Wo
    wvo = np.ascontiguousarray(wvo64.astype(bf))
    g1 = Wg[:D, 0]
    g2 = wvo64 @ Wg[D:, 0]
    ones_rD = np.full(D, 1.0 / D)
    gq = np.ascontiguousarray(np.stack([-g1, ones_rD], axis=1).astype(bf))
    gm = np.ascontiguousarray(
        np.stack([-g2, wvo64.sum(axis=1) / D], axis=1).astype(bf)
    )
    ident = np.eye(P, dtype=bf)

    if "nc" not in _CACHE:
        _CACHE["nc"] = _build()
    nc = _CACHE["nc"]

    qm = np.empty((B, (K + 1) * D), dtype=bf)
    qm[:, :D] = query
    qm[:, D:] = mem.reshape(B, K * D)
    maskf = np.ascontiguousarray(mask.astype(np.float32))
    conf = (1.0 / (1.0 + np.exp(-(sims.max(axis=-1) - SIM_THRESH)))).astype(
        np.float32
    ).reshape(B, 1)
    in_maps = []
    for c in range(N_CORES):
        sl = slice(c * ROWS, (c + 1) * ROWS)
        in_maps.append({
            "qm": qm[sl], "maskf": maskf[sl], "conf": conf[sl],
            "wqk": wqk, "wvo": wvo, "gq": gq, "gm": gm, "ident": ident,
        })

    from concourse.bass_utils import run_bass_kernel_spmd

    res = run_bass_kernel_spmd(nc, in_maps, list(range(N_CORES)), trace=TRACE)
    LAST_RESULTS = res
    return np.concatenate(
        [res.results[c]["o"].astype(np.float32) for c in range(N_CORES)], axis=0
    )
